# revision 7
# baseline (speedup 1.0000x reference)
"""Trainium2 Bass kernel for nn_CrossAttention (b=4, n=2048, j=2048, h=8, d=64).

Sharding: 8 cores = (batch 4) x (query-half 2). Each core computes all 8 heads
for 1024 query rows of one batch.

Strategy:
- Host-side mask gather: masked context rows get exactly 0 softmax weight, so
  only the unmasked rows (~half) are shipped/projected/attended. Exact math.
- Host-side transposes + pre-tiled DMA layouts: every input lands as
  [128, bytes] with per-partition-contiguous lines (single fat descriptors),
  split across the two hardware DGE queues (SP + ACT).
- Row-tiled S matmuls: even/odd heads of a pair live on PE quadrant rows 0/64;
  adjacent S matmuls target disjoint row-groups and overlap in the array.
- Single-pass weave: Q/K/V projection and out-projection matmul units are
  interleaved into the exp(ACT)-bound attention stream so the PE never idles.

Per-core pipeline:
  qT = Wq^T @ xT  (xT from host)         [inner, n]    fp16
  kT = Wk^T @ ctxT (ctxT packed, host)   [inner, Jp]   fp16
  v  = ctxT^T @ Wv -> vaug fp16          [Jp, h, d+1]  (ones col = denominator)
  per (ng, head-pair, jb):  S-pair[jb, n-half x 2 heads] (row-tiled, K=64)
        pt = exp(0.125*S + padbias)  (ACT)  -> fp16
        avp[h] += vaug_h^T @ pt_h  (accum over jb)
  normalize by row d (denominator), out = oT^T @ Wo + b_o -> DMA
"""
import math
import numpy as np
from contextlib import ExitStack

from concourse import bacc, mybir, tile
from concourse.bass_utils import run_bass_kernel_spmd

F32 = mybir.dt.float32
F32R = mybir.dt.float32r
F16 = mybir.dt.float16

HEADS = 8
D = 64
N_CORE = 1024   # query rows per core
CQ = 1024       # query_dim
CK = 768        # context_dim
INNER = 512
OUT_D = 1024
P = 128
SCALE = 0.125
MASK_NEG = -30.0

KQ = CQ // P          # 8
KC = CK // P          # 6
DB = INNER // P       # 4
NG = N_CORE // 512    # 2


def build_nc(JB):
    JP = JB * P
    KGRP = (JP + 511) // 512  # ragged 512-col groups over JP

    nc = bacc.Bacc("TRN2", target_bir_lowering=False)
    # All inputs host-pre-tiled to [128, L]: partition p's data contiguous.
    xT_d = nc.dram_tensor("xT", [P, KQ * N_CORE], F16, kind="ExternalInput")
    ctxT_d = nc.dram_tensor("ctxT", [P, KC * JP], F16, kind="ExternalInput")
    mb_d = nc.dram_tensor("mb", [P, JB], F32, kind="ExternalInput")
    wq_d = nc.dram_tensor("wq", [P, KQ * INNER], F16, kind="ExternalInput")
    wk_d = nc.dram_tensor("wk", [P, KC * INNER], F16, kind="ExternalInput")
    wv_d = nc.dram_tensor("wv", [P, KC * INNER], F16, kind="ExternalInput")
    wo_d = nc.dram_tensor("wo", [P, DB * OUT_D], F32, kind="ExternalInput")
    bo_d = nc.dram_tensor("bo", [1, OUT_D], F32, kind="ExternalInput")
    out_d = nc.dram_tensor("out", [N_CORE, OUT_D], F32, kind="ExternalOutput")

    with ExitStack() as top:
        tc = top.enter_context(tile.TileContext(nc))
        persist = top.enter_context(tc.tile_pool(name="persist", bufs=1))

        xT_sb = persist.tile([P, KQ, N_CORE], F16, name="xT")
        ctxT_sb = persist.tile([P, KC, JP], F16, name="ctxT")
        wq_sb = persist.tile([P, KQ, INNER], F16, name="wq")
        wk_sb = persist.tile([P, KC, INNER], F16, name="wk")
        wv_sb = persist.tile([P, KC, INNER], F16, name="wv")
        wo_sb = persist.tile([P, DB, OUT_D], F32R, name="wo")
        mb_sb = persist.tile([P, JB], F32, name="mb")
        bo_sb = persist.tile([1, OUT_D], F32, name="bo")
        b_bc = persist.tile([P, OUT_D], F32, name="b_bc")
        qT = persist.tile([P, DB, N_CORE], F16, name="qT")
        kT = persist.tile([P, DB, JP], F16, name="kT")
        vaug = persist.tile([P, JB, HEADS, D + 1], F16, name="vaug")
        oT = persist.tile([P, DB, N_CORE], F32R, name="oT")

        # --- DMAs: earliest-needed first, split across SP(sync) + ACT(scalar)
        # hardware DGE queues. Pre-tiled layouts -> fat contiguous descriptors.
        H_XT = KQ * N_CORE // 2
        H_CT = KC * JP // 2
        nc.sync.dma_start(out=xT_sb[:, 0:KQ // 2, :], in_=xT_d[:, 0:H_XT])
        nc.scalar.dma_start(out=xT_sb[:, KQ // 2:KQ, :], in_=xT_d[:, H_XT:])
        nc.scalar.dma_start(out=wq_sb, in_=wq_d[:, :])
        nc.sync.dma_start(out=ctxT_sb[:, 0:KC // 2, :], in_=ctxT_d[:, 0:H_CT])
        nc.scalar.dma_start(out=ctxT_sb[:, KC // 2:KC, :], in_=ctxT_d[:, H_CT:])
        nc.sync.dma_start(out=wk_sb, in_=wk_d[:, :])
        nc.scalar.dma_start(out=mb_sb, in_=mb_d[:, :])
        nc.sync.dma_start(out=wv_sb, in_=wv_d[:, :])
        nc.scalar.dma_start(out=bo_sb, in_=bo_d[:, :])
        nc.sync.dma_start(out=wo_sb[:, 0:2, :], in_=wo_d[:, 0:2 * OUT_D].bitcast(F32R))
        nc.scalar.dma_start(out=wo_sb[:, 2:4, :], in_=wo_d[:, 2 * OUT_D:].bitcast(F32R))
        nc.gpsimd.partition_broadcast(b_bc, bo_sb)

        ps_s = top.enter_context(tc.tile_pool(name="ps_s", bufs=2, space="PSUM"))
        ps_av = top.enter_context(tc.tile_pool(name="ps_av", bufs=3, space="PSUM"))
        ps_w = top.enter_context(tc.tile_pool(name="ps_w", bufs=1, space="PSUM"))
        ptp = top.enter_context(tc.tile_pool(name="ptp", bufs=4))
        small = top.enter_context(tc.tile_pool(name="small", bufs=4))
        outp = top.enter_context(tc.tile_pool(name="outp", bufs=3))

        # --- work units (each: one psum round-trip) ---
        def emit_qp(db, ng):
            qp = ps_w.tile([P, 512], F32, name="wp")
            for kc in range(KQ):
                nc.tensor.matmul(
                    qp,
                    wq_sb[:, kc, db * P:(db + 1) * P],
                    xT_sb[:, kc, ng * 512:(ng + 1) * 512],
                    start=(kc == 0), stop=(kc == KQ - 1),
                )
            nc.vector.tensor_copy(out=qT[:, db, ng * 512:(ng + 1) * 512], in_=qp)

        def emit_kp(db, jg):
            w = min(512, JP - jg * 512)
            kp = ps_w.tile([P, 512], F32, name="wp")
            for kc in range(KC):
                nc.tensor.matmul(
                    kp[:, 0:w],
                    wk_sb[:, kc, db * P:(db + 1) * P],
                    ctxT_sb[:, kc, jg * 512:jg * 512 + w],
                    start=(kc == 0), stop=(kc == KC - 1),
                )
            nc.vector.tensor_copy(out=kT[:, db, jg * 512:jg * 512 + w], in_=kp[:, 0:w])

        def emit_vp(jb):
            vp = ps_w.tile([P, 512], F32, name="wp")
            for kc in range(KC):
                nc.tensor.matmul(
                    vp,
                    ctxT_sb[:, kc, jb * P:(jb + 1) * P],
                    wv_sb[:, kc, :],
                    start=(kc == 0), stop=(kc == KC - 1),
                )
            nc.vector.tensor_copy(
                out=vaug[:, jb, :, 0:D],
                in_=vp.rearrange("p (h d) -> p h d", h=HEADS),
            )
            nc.vector.memset(vaug[:, jb, :, D:D + 1], 1.0)

        def emit_out(ng, nb, og, pool=None):
            ncol = ng * 512 + nb * P
            if pool is None:
                op = ps_w.tile([P, 512], F32, name="wp")
            else:
                # reuse the freed S slots: same name/shape, slice half
                op = pool.tile([P, 1024], F32, name="sp")[:, 0:512]
            for t in range(DB):
                nc.tensor.matmul(
                    op,
                    oT[:, t, ncol:ncol + P],
                    wo_sb[:, t, og * 512:(og + 1) * 512],
                    start=(t == 0), stop=(t == DB - 1),
                )
            ob = outp.tile([P, 512], F32, name="ob")
            nc.vector.tensor_add(ob, op, b_bc[:, og * 512:(og + 1) * 512])
            eng = nc.sync if og == 0 else nc.scalar
            eng.dma_start(
                out=out_d[ncol:ncol + P, og * 512:(og + 1) * 512], in_=ob
            )

        # --- filler schedule: (ng, hp) block -> iter -> unit closures ---
        blocks = {}

        def add_unit(ng, hp, it, fn):
            it = max(0, min(it, JB - 1))
            blocks.setdefault((ng, hp), {}).setdefault(it, []).append(fn)

        # VP 0..2 in pre-phase; rest early in block (0,0), ahead of their AV use
        for j in range(3, JB):
            add_unit(0, 0, j - 3, lambda j=j: emit_vp(j))
        # KP db=1..3 inside block (0, db-1); all must precede block (0, db)
        for db in range(1, DB):
            for jg in range(KGRP):
                add_unit(0, db - 1, 2 + 2 * jg, lambda db=db, jg=jg: emit_kp(db, jg))
        # QP (db, ng=0) before block (0, db); (db, ng=1) before block (1, db)
        for db in range(1, DB):
            add_unit(0, db - 1, JB - 2, lambda db=db: emit_qp(db, 0))
        add_unit(0, DB - 1, 0, lambda: emit_qp(0, 1))
        for db in range(1, DB):
            add_unit(1, db - 1, 0, lambda db=db: emit_qp(db, 1))
        # OUT for ng=0 woven into ng=1 blocks
        for nb in range(2):
            for og in range(2):
                add_unit(1, 0, 1 + 2 * nb + og, lambda nb=nb, og=og: emit_out(0, nb, og))
        for nb in range(2, 4):
            for og in range(2):
                add_unit(1, 1, 1 + 2 * (nb - 2) + og,
                         lambda nb=nb, og=og: emit_out(0, nb, og))

        # --- pre-phase: minimal work to start attention ---
        emit_qp(0, 0)
        for jg in range(KGRP):
            emit_kp(0, jg)
        for j in range(min(3, JB)):
            emit_vp(j)

        # --- attention ---
        def normalize(avp, hp, par, ng):
            l_sb = small.tile([1, 512], F32, name="l_sb")
            nc.vector.tensor_copy(out=l_sb, in_=avp[D:D + 1, :])
            r_f = small.tile([1, 512], F32, name="r_f")
            nc.vector.reciprocal_approx_fast(r_f, l_sb)
            bc_sb = small.tile([D, 512], F32, name="bc_sb")
            nc.gpsimd.partition_broadcast(bc_sb, r_f)
            nc.vector.tensor_mul(
                oT[D * par:D * par + D, hp, ng * 512:(ng + 1) * 512],
                avp[0:D, :],
                bc_sb,
            )

        for ng in range(NG):
            for hp in range(DB):
                ext = blocks.get((ng, hp), {})
                avp = [
                    ps_av.tile([D + 1, 512], F32, name="av") for _ in range(2)
                ]
                pts = {}

                def av_pair(jb, hp=hp, avp=avp, pts=pts):
                    for par in range(2):
                        nc.tensor.matmul(
                            avp[par],
                            vaug[:, jb, 2 * hp + par, :],
                            pts[jb][:, par * 512:(par + 1) * 512],
                            start=(jb == 0), stop=(jb == JB - 1),
                        )

                for jb in range(JB):
                    sp = ps_s.tile([P, 1024], F32, name="sp")
                    for par in range(2):
                        nc.tensor.matmul(
                            sp[:, par * 512:(par + 1) * 512],
                            kT[D * par:D * par + D, hp, jb * P:(jb + 1) * P],
                            qT[D * par:D * par + D, hp, ng * 512:(ng + 1) * 512],
                            start=True, stop=True,
                        )
                    pt = ptp.tile([P, 1024], F16, name="pt")
                    nc.scalar.activation(
                        out=pt, in_=sp,
                        func=mybir.ActivationFunctionType.Exp,
                        bias=mb_sb[:, jb:jb + 1], scale=SCALE,
                    )
                    pts[jb] = pt
                    for fn in ext.get(jb, []):
                        fn()
                    if jb > 0:
                        av_pair(jb - 1)
                av_pair(JB - 1)
                for par in range(2):
                    normalize(avp[par], hp, par, ng)

        # --- tail: out-proj for ng=1, 3-slot psum rotation (ps_s is free) ---
        for i, (nb, og) in enumerate((nb, og) for nb in range(4) for og in range(2)):
            emit_out(1, nb, og, pool=(ps_s if i % 2 == 0 else None))

    nc.finalize()
    return nc


_NC_CACHE = {}


def _get_nc(JB):
    if JB not in _NC_CACHE:
        _NC_CACHE[JB] = build_nc(JB)
    return _NC_CACHE[JB]


def _tile_rows(a, dtype):
    """[R, C] -> [128, (R//128)*C]: row r=c*128+p lands at partition p, chunk c."""
    r, c = a.shape
    return np.ascontiguousarray(
        a.reshape(r // P, P, c).transpose(1, 0, 2).reshape(P, (r // P) * c)
    ).astype(dtype)


def prepare(x, context, mask, W_q, W_k, W_v, W_o, b_o):
    x = np.asarray(x, dtype=np.float32)
    context = np.asarray(context, dtype=np.float32)
    mask = np.asarray(mask).astype(bool)
    counts = mask.sum(axis=1)
    JB = max(1, int(math.ceil(counts.max() / P)))
    JP = JB * P

    shared = {
        "wq": _tile_rows(np.asarray(W_q, np.float32), np.float16),
        "wk": _tile_rows(np.asarray(W_k, np.float32), np.float16),
        "wv": _tile_rows(np.asarray(W_v, np.float32), np.float16),
        "wo": _tile_rows(np.asarray(W_o, np.float32), np.float32),
        "bo": np.ascontiguousarray(
            np.asarray(b_o, dtype=np.float32).reshape(1, OUT_D)
        ),
    }
    per_batch = []
    for bi in range(4):
        idx = np.flatnonzero(mask[bi])
        ne = len(idx)
        ctxp = np.zeros((JP, CK), dtype=np.float32)
        ctxp[:ne] = context[bi][idx]
        ctxT = _tile_rows(np.ascontiguousarray(ctxp.T), np.float16)
        mb = np.full(JP, MASK_NEG, dtype=np.float32)
        mb[:ne] = 0.0
        mb = np.ascontiguousarray(mb.reshape(JB, P).T)  # [128, JB]
        per_batch.append((ctxT, mb))

    in_maps = []
    for c in range(8):
        bi, nh = c // 2, c % 2
        ctxT, mb = per_batch[bi]
        xT = _tile_rows(
            np.ascontiguousarray(x[bi, nh * N_CORE:(nh + 1) * N_CORE].T),
            np.float16,
        )
        in_maps.append({"xT": xT, "ctxT": ctxT, "mb": mb, **shared})
    return _get_nc(JB), in_maps


def kernel(x, context, mask, W_q, W_k, W_v, W_o, b_o):
    nc, in_maps = prepare(x, context, mask, W_q, W_k, W_v, W_o, b_o)
    res = run_bass_kernel_spmd(nc, in_maps, core_ids=list(range(8)))
    out = np.empty((4, 2048, OUT_D), dtype=np.float32)
    for c in range(8):
        bi, nh = c // 2, c % 2
        out[bi, nh * N_CORE:(nh + 1) * N_CORE] = res.results[c]["out"]
    return out
